# revision 25
# baseline (speedup 1.0000x reference)
"""Trainium2 Bass kernel for AtomTransformerBlock (block-sparse attention).

8 NeuronCores, atoms sharded 256 queries/core.  Key ideas:
  - Only 1/16 of pair_emb is needed: indirect-DMA gather of the
    (i, block_index[i,b]) 64B entries into a count-sorted rectangular
    [key-slot, slot-entry] layout (per-core slot permutation).
  - Attention computed densely per core in transposed [key, query] layout:
    dense scores on PE, exp fused into the PSUM drain on ACT, sparse
    count*exp(pair_bias) mask built with gpsimd local_scatter, attn@V and
    softmax denominator as plain matmuls.
  - k/v replicated on every core from full x with layernorm folded into
    the projection as per-atom scale + rank-1 mean correction.
"""

import math
from contextlib import ExitStack

import numpy as np

N = 2048
C = 128
CP = 16
H = 4
DH = 32
EPS = 1e-5
NCORES = 8
OWN = N // NCORES          # 256 queries per core
NCH = N // 128             # 16 key chunks
QT = OWN // 128            # 2 query tiles
SCALE = 1.0 / math.sqrt(DH)


# --------------------------------------------------------------------------
# Host-side preprocessing
# --------------------------------------------------------------------------

def host_prep(block_index):
    bi = np.asarray(block_index).astype(np.int64)
    percore = []
    for r in range(NCORES):
        rows = bi[r * OWN:(r + 1) * OWN]
        pairs_by_j = [[] for _ in range(N)]
        for q in range(OWN):
            vals, cnts = np.unique(rows[q], return_counts=True)
            for j, c in zip(vals.tolist(), cnts.tolist()):
                pairs_by_j[j].append((q, c))
        njcnt = np.array([len(p) for p in pairs_by_j])
        order = np.argsort(-njcnt, kind="stable")
        percore.append((pairs_by_j, order, njcnt))

    S = []
    for c in range(NCH):
        w = 1
        for pairs_by_j, order, njcnt in percore:
            w = max(w, int(njcnt[order[c * 128:(c + 1) * 128]].max()))
        S.append(w + (w % 2))
    Stot = sum(S)
    offs = np.cumsum([0] + S[:-1]).tolist()

    tensors = []
    for r in range(NCORES):
        pairs_by_j, order, njcnt = percore[r]
        poff = np.zeros((128, Stot), np.int32)
        pcnt = np.zeros((128, Stot), np.float32)
        pqt = np.full((128, 4 * Stot), -1, np.int16)
        for c in range(NCH):
            o = offs[c]
            for p in range(128):
                j = int(order[c * 128 + p])
                for s, (q, cnt) in enumerate(pairs_by_j[j]):
                    poff[p, o + s] = q * N + j
                    pcnt[p, o + s] = cnt
                    for h in range(H):
                        pqt[p, 4 * (o + s) + h] = 4 * q + h
        pioff = np.ascontiguousarray(
            order.astype(np.int32).reshape(NCH, 128).T)      # [128, NCH]
        tensors.append(dict(poff=poff, pcnt=pcnt, pqt=pqt, pioff=pioff))
    return S, offs, Stot, tensors


# --------------------------------------------------------------------------
# Device program (identical for all cores; per-core values differ)
# --------------------------------------------------------------------------

def build_nc(S, offs, Stot):
    import os
    import concourse.bass as bass
    import concourse.tile as tile
    from concourse import bacc, mybir
    from concourse.masks import make_identity

    fp32 = mybir.dt.float32
    bf16 = mybir.dt.bfloat16
    i32 = mybir.dt.int32
    i16 = mybir.dt.int16
    AF = mybir.ActivationFunctionType
    OP = mybir.AluOpType
    AX = mybir.AxisListType

    nc = bacc.Bacc()

    x_d = nc.dram_tensor("x", [N, C], fp32, kind="ExternalInput")
    xown_d = nc.dram_tensor("xown", [OWN, C], fp32, kind="ExternalInput")
    pair_d = nc.dram_tensor("pair", [OWN * N, CP], fp32, kind="ExternalInput")
    Wq_d = nc.dram_tensor("Wq", [C, C], fp32, kind="ExternalInput")
    Wk_d = nc.dram_tensor("Wk", [C, C], fp32, kind="ExternalInput")
    Wv_d = nc.dram_tensor("Wv", [C, C], fp32, kind="ExternalInput")
    Wo_d = nc.dram_tensor("Wout", [C, C], fp32, kind="ExternalInput")
    W1_d = nc.dram_tensor("W1", [4 * C, C], fp32, kind="ExternalInput")
    W2_d = nc.dram_tensor("W2", [C, 4 * C], fp32, kind="ExternalInput")
    b1_d = nc.dram_tensor("b1", [4 * C], fp32, kind="ExternalInput")
    b2_d = nc.dram_tensor("b2", [C], fp32, kind="ExternalInput")
    Wb_d = nc.dram_tensor("Wb", [H], fp32, kind="ExternalInput")
    pioff_d = nc.dram_tensor("pioff", [128, NCH], i32, kind="ExternalInput")
    poff_d = nc.dram_tensor("poff", [128, Stot], i32, kind="ExternalInput")
    pcnt_d = nc.dram_tensor("pcnt", [128, Stot], fp32, kind="ExternalInput")
    pqt_d = nc.dram_tensor("pqt", [128, 4 * Stot], i16, kind="ExternalInput")
    out_d = nc.dram_tensor("out", [OWN, C], fp32, kind="ExternalOutput")

    xstage_d = nc.dram_tensor("xstage", [N, C], bf16, kind="Internal")
    zstage_d = nc.dram_tensor("zstage", [H * OWN], fp32, kind="Internal")

    _PH = ["wload", "xprep", "stats", "proj", "pair", "mask", "attn",
           "post", "tail"]
    _stop = os.environ.get("KSTOP", "tail")
    _lim = _PH.index(_stop)

    def on(ph):
        return _PH.index(ph) <= _lim

    with tile.TileContext(nc) as tc, ExitStack() as ctx:
        P = ctx.enter_context(tc.tile_pool(name="per", bufs=1))
        WK = ctx.enter_context(tc.tile_pool(name="wk", bufs=2))
        PSB = ctx.enter_context(tc.tile_pool(name="psb", bufs=2, space="PSUM"))
        PSA = ctx.enter_context(tc.tile_pool(name="psa", bufs=1, space="PSUM"))

        ident = P.tile([128, 128], fp32, tag="ident")
        make_identity(nc, ident[:])
        identb = P.tile([128, 128], bf16, tag="identb")
        nc.vector.tensor_copy(out=identb[:], in_=ident[:])
        epscol = P.tile([128, 1], fp32, tag="epscol")
        nc.vector.memset(epscol[:], EPS)
        pioff_t = P.tile([128, NCH], i32, tag="pioff")
        nc.sync.dma_start(out=pioff_t[:], in_=pioff_d[:])
        poff_t = P.tile([128, Stot], i32, tag="poff")
        nc.sync.dma_start(out=poff_t[:], in_=poff_d[:])
        pcnt_t = P.tile([128, Stot], fp32, tag="pcnt")
        nc.sync.dma_start(out=pcnt_t[:], in_=pcnt_d[:])
        pqt_t = P.tile([128, 4 * Stot], i16, tag="pqt")
        nc.sync.dma_start(out=pqt_t[:], in_=pqt_d[:])

        # ---------------- weights ----------------
        def load_t(dram, tag, rs=None, cs=None):
            rs = rs or slice(0, dram.shape[0])
            cs = cs or slice(0, dram.shape[1])
            raw = WK.tile([128, 128], fp32, tag="wraw")
            nc.sync.dma_start(out=raw[:], in_=dram[rs, cs])
            ps = PSB.tile([128, 128], fp32, tag="big", name="wps")
            nc.tensor.transpose(out=ps[:], in_=raw[:], identity=ident[:])
            return ps

        def drain_t(ps, dst_ap):
            nc.vector.tensor_copy(out=dst_ap, in_=ps[:])

        Wqt = P.tile([128, 128], bf16, tag="Wqt")
        drain_t(load_t(Wq_d, "Wqt"), Wqt[:])
        Wkt = P.tile([128, 128], bf16, tag="Wkt")
        drain_t(load_t(Wk_d, "Wkt"), Wkt[:])
        Wvt = P.tile([128, 128], bf16, tag="Wvt")
        drain_t(load_t(Wv_d, "Wvt"), Wvt[:])
        Wot = P.tile([128, 128], bf16, tag="Wot")
        drain_t(load_t(Wo_d, "Wot"), Wot[:])
        w1t = P.tile([128, 4, 128], bf16, tag="w1t")     # [cin, kchunk, m]
        for k in range(4):
            drain_t(load_t(W1_d, "w1t", rs=slice(128 * k, 128 * (k + 1))),
                    w1t[:, k, :])
        w2t = P.tile([128, 4, 128], bf16, tag="w2t")     # [m, kchunk, cout]
        for k in range(4):
            drain_t(load_t(W2_d, "w2t", cs=slice(128 * k, 128 * (k + 1))),
                    w2t[:, k, :])

        b1c = P.tile([128, 4], fp32, tag="b1c")
        nc.sync.dma_start(out=b1c[:], in_=b1_d[:].rearrange("(k p) -> p k", p=128))
        b2raw = P.tile([1, C], fp32, tag="b2raw")
        nc.sync.dma_start(out=b2raw[:], in_=b2_d[:].rearrange("(o c) -> o c", o=1))
        b2row = P.tile([1, C], bf16, tag="b2row")
        nc.vector.tensor_copy(out=b2row[:], in_=b2raw[:])

        ones_col = P.tile([128, 1], bf16, tag="ones_col")
        nc.vector.memset(ones_col[:], 1.0)
        ones_row = P.tile([1, 128], bf16, tag="ones_row")
        nc.vector.memset(ones_row[:], 1.0)
        ones4 = P.tile([128, 4], bf16, tag="ones4")
        nc.vector.memset(ones4[:], 1.0)

        # Wb broadcast to all partitions via rank-1 matmul
        wbraw = P.tile([1, H], fp32, tag="wbraw")
        nc.sync.dma_start(out=wbraw[:], in_=Wb_d[:].rearrange("(o h) -> o h", o=1))
        wbrow = P.tile([1, H], bf16, tag="wbrow")
        nc.vector.tensor_copy(out=wbrow[:], in_=wbraw[:])
        wb_ps = PSB.tile([128, H], fp32, tag="big", name="wbps")
        nc.tensor.matmul(wb_ps[:], lhsT=ones_row[:], rhs=wbrow[:],
                         start=True, stop=True)
        wbbc = P.tile([128, H], fp32, tag="wbbc")
        nc.vector.tensor_copy(out=wbbc[:], in_=wb_ps[:])

        outf = P.tile([128, QT, C], fp32, tag="outf")
        nc.vector.memset(outf[:], 0.0)

        # ---------------- x gather (slot order) + transpose ----------------
        if not on("xprep"):
            nc.sync.dma_start(out=out_d[:].rearrange("(t p) c -> p t c", p=128),
                              in_=outf[:])
            nc.compile()
            return nc
        xpi = P.tile([128, NCH, C], fp32, tag="xpi")
        nc.gpsimd.indirect_dma_start(
            out=xpi[:], out_offset=None,
            in_=x_d[:],
            in_offset=bass.IndirectOffsetOnAxis(ap=pioff_t[:], axis=0))
        xpib = P.tile([128, NCH, C], bf16, tag="xpib")
        nc.vector.tensor_copy(out=xpib[:], in_=xpi[:])
        xT = P.tile([128, N], bf16, tag="xT")
        for c in range(NCH):
            xtp = PSB.tile([128, 128], bf16, tag="big", name="xtp")
            nc.tensor.transpose(out=xtp[:], in_=xpib[:, c, :],
                                identity=identb[:])
            nc.scalar.copy(out=xT[:, 128 * c:128 * (c + 1)], in_=xtp[:])

        # ---------------- own-row LN -> hT ----------------
        xow = P.tile([128, QT, C], fp32, tag="xow")
        nc.sync.dma_start(out=xow[:],
                          in_=xown_d[:].rearrange("(t p) c -> p t c", p=128))
        hT = P.tile([128, OWN], bf16, tag="hT")
        for t in range(QT):
            mu = WK.tile([128, 1], fp32, tag="mu")
            nc.vector.tensor_reduce(out=mu[:], in_=xow[:, t, :], axis=AX.X,
                                    op=OP.add)
            nc.vector.tensor_scalar_mul(mu[:], mu[:], 1.0 / C)
            xc = WK.tile([128, C], fp32, tag="xc")
            nc.vector.tensor_scalar_sub(xc[:], xow[:, t, :], mu[:])
            sq = WK.tile([128, C], fp32, tag="sq")
            var = WK.tile([128, 1], fp32, tag="var")
            nc.scalar.activation(sq[:], xc[:], AF.Square, accum_out=var[:])
            std = WK.tile([128, 1], fp32, tag="std")
            nc.scalar.activation(std[:], var[:], AF.Sqrt, bias=epscol[:], scale=1.0 / C)
            rstd = WK.tile([128, 1], fp32, tag="rstd")
            nc.vector.reciprocal(rstd[:], std[:])
            hloc = WK.tile([128, C], bf16, tag="hloc")
            nc.vector.tensor_scalar_mul(hloc[:], xc[:], rstd[:])
            hps = PSB.tile([128, 128], bf16, tag="big", name="hps")
            nc.tensor.transpose(out=hps[:], in_=hloc[:], identity=identb[:])
            nc.vector.tensor_copy(out=hT[:, 128 * t:128 * (t + 1)], in_=hps[:])

        if not on("stats"):
            nc.sync.dma_start(out=out_d[:].rearrange("(t p) c -> p t c", p=128),
                              in_=outf[:])
            nc.compile()
            return nc
        # ---------------- per-atom stats (slot order) ----------------
        xTsq = P.tile([128, N], bf16, tag="xTsq")
        nc.vector.tensor_tensor(out=xTsq[:], in0=xT[:], in1=xT[:], op=OP.mult)
        st_ps = PSB.tile([128, 2 * NCH], fp32, tag="big", name="stps")
        for c in range(NCH):
            sl = slice(128 * c, 128 * (c + 1))
            nc.tensor.matmul(st_ps[:, c:c + 1], lhsT=xT[:, sl], rhs=ones_col[:],
                             start=True, stop=True)
            nc.tensor.matmul(st_ps[:, NCH + c:NCH + c + 1], lhsT=xTsq[:, sl],
                             rhs=ones_col[:], start=True, stop=True)
        mucol = P.tile([128, NCH], fp32, tag="mucol")
        nc.vector.tensor_scalar_mul(mucol[:], st_ps[:, :NCH], 1.0 / C)
        varcol = WK.tile([128, NCH], fp32, tag="varcol")
        nc.vector.tensor_tensor(out=varcol[:], in0=mucol[:], in1=mucol[:],
                                op=OP.mult)
        x2col = WK.tile([128, NCH], fp32, tag="x2col")
        nc.vector.tensor_scalar_mul(x2col[:], st_ps[:, NCH:], 1.0 / C)
        nc.vector.tensor_tensor(out=varcol[:], in0=x2col[:], in1=varcol[:],
                                op=OP.subtract)
        stdc = WK.tile([128, NCH], fp32, tag="stdc")
        nc.scalar.activation(stdc[:], varcol[:], AF.Sqrt, bias=epscol[:])
        rstdc = P.tile([128, NCH], fp32, tag="rstdc")
        nc.vector.reciprocal(rstdc[:], stdc[:])
        rstdsc = P.tile([128, NCH], fp32, tag="rstdsc")
        nc.vector.tensor_scalar_mul(rstdsc[:], rstdc[:], SCALE)

        negmu_b = WK.tile([128, NCH], bf16, tag="negmu_b")
        nc.vector.tensor_scalar_mul(negmu_b[:], mucol[:], -1.0)
        nmt_ps = PSB.tile([NCH, 128], bf16, tag="big", name="nmtps")
        nc.tensor.transpose(out=nmt_ps[:], in_=negmu_b[:], identity=identb[:])
        nmt = WK.tile([NCH, 128], bf16, tag="nmts")
        nc.vector.tensor_copy(out=nmt[:], in_=nmt_ps[:])
        negmuf = P.tile([1, N], bf16, tag="negmuf")
        nc.sync.dma_start(out=negmuf[:].rearrange("o (c p) -> o c p", p=128),
                          in_=nmt[:].rearrange("c (o p) -> c o p", o=1))

        wr_ps = PSB.tile([1, 2 * C], fp32, tag="big", name="wrps")
        nc.tensor.matmul(wr_ps[:, :C], lhsT=ones_col[:], rhs=Wkt[:],
                         start=True, stop=True)
        nc.tensor.matmul(wr_ps[:, C:], lhsT=ones_col[:], rhs=Wvt[:],
                         start=True, stop=True)
        wrrow = P.tile([1, 2 * C], bf16, tag="wrrow")
        nc.vector.tensor_copy(out=wrrow[:], in_=wr_ps[:])

        if not on("proj"):
            nc.sync.dma_start(out=out_d[:].rearrange("(t p) c -> p t c", p=128),
                              in_=outf[:])
            nc.compile()
            return nc
        # ---------------- projections ----------------
        Pk = P.tile([128, N], bf16, tag="Pk")
        for nk in range(N // 512):
            sl = slice(512 * nk, 512 * (nk + 1))
            ps = PSB.tile([128, 512], fp32, tag="big", name="pkps")
            nc.tensor.matmul(ps[:], lhsT=Wkt[:], rhs=xT[:, sl],
                             start=True, stop=False)
            nc.tensor.matmul(ps[:], lhsT=wrrow[:, :C], rhs=negmuf[:, sl],
                             start=False, stop=True)
            nc.scalar.copy(out=Pk[:, sl], in_=ps[:])

        qT = P.tile([128, OWN], bf16, tag="qT")
        qps = PSB.tile([128, OWN], fp32, tag="big", name="qps")
        nc.tensor.matmul(qps[:], lhsT=Wqt[:], rhs=hT[:], start=True, stop=True)
        nc.vector.tensor_copy(out=qT[:], in_=qps[:])
        # per-head zero-padded copies so score matmuls can use full K=128
        qT4 = P.tile([128, H, OWN], bf16, tag="qT4")
        nc.vector.memset(qT4[:], 0.0)
        for h in range(H):
            hs = slice(DH * h, DH * (h + 1))
            nc.vector.tensor_copy(out=qT4[hs, h, :], in_=qT[hs, :])

        vsb = P.tile([128, NCH, H, DH + 1], bf16, tag="vsb")
        nc.vector.memset(vsb[:], 1.0)
        for c in range(NCH):
            sl = slice(128 * c, 128 * (c + 1))
            ps = PSB.tile([128, C], fp32, tag="big", name="vps")
            nc.tensor.matmul(ps[:], lhsT=xT[:, sl], rhs=Wvt[:],
                             start=True, stop=False)
            nc.tensor.matmul(ps[:], lhsT=negmuf[:, sl], rhs=wrrow[:, C:],
                             start=False, stop=True)
            nc.vector.tensor_scalar_mul(
                vsb[:, c, :, :DH],
                ps[:].rearrange("p (h d) -> p h d", h=H),
                rstdc[:, c:c + 1])

        if not on("pair"):
            nc.sync.dma_start(out=out_d[:].rearrange("(t p) c -> p t c", p=128),
                              in_=outf[:])
            nc.compile()
            return nc
        # ---------------- pair gather + mask ----------------
        pmraw = P.tile([128, Stot, CP], fp32, tag="pmraw")
        wdata = P.tile([128, Stot, H], bf16, tag="wdata")
        wdataf = wdata[:].rearrange("p s h -> p (s h)")
        GC = 4
        bnd = [0]
        for i in range(1, GC):
            tgt = i * Stot / GC
            bnd.append(min(range(NCH + 1),
                           key=lambda c: abs((offs[c] if c < NCH else Stot) - tgt)))
        bnd.append(NCH)
        grp = []
        for g in range(GC):
            lo = offs[bnd[g]]
            hi = offs[bnd[g + 1]] if bnd[g + 1] < NCH else Stot
            grp.append((lo, hi))
            if lo == hi:
                continue
            nc.gpsimd.indirect_dma_start(
                out=pmraw[:, lo:hi, :], out_offset=None,
                in_=pair_d[:],
                in_offset=bass.IndirectOffsetOnAxis(ap=poff_t[:, lo:hi], axis=0))
            w = hi - lo
            pm = WK.tile([128, Stot // 2], fp32, tag="pm")
            nc.vector.tensor_reduce(out=pm[:, :w], in_=pmraw[:, lo:hi, :],
                                    axis=AX.X, op=OP.add)
            pmw = WK.tile([128, Stot // 2, H], fp32, tag="pmw")
            nc.vector.tensor_tensor(
                out=pmw[:, :w, :],
                in0=pm[:, :w].rearrange("p (s u) -> p s u", u=1)
                .to_broadcast([128, w, H]),
                in1=wbbc[:].rearrange("p (o h) -> p o h", o=1)
                .to_broadcast([128, w, H]),
                op=OP.mult)
            expw = WK.tile([128, Stot // 2, H], fp32, tag="expw")
            nc.scalar.activation(expw[:, :w, :], pmw[:, :w, :], AF.Exp,
                                 scale=1.0 / CP)
            nc.vector.tensor_tensor(
                out=wdata[:, lo:hi, :],
                in0=expw[:, :w, :],
                in1=pcnt_t[:, lo:hi].rearrange("p (s u) -> p s u", u=1)
                .to_broadcast([128, w, H]),
                op=OP.mult)

        if not on("mask"):
            nc.sync.dma_start(out=out_d[:].rearrange("(t p) c -> p t c", p=128),
                              in_=outf[:])
            nc.compile()
            return nc
        cbt = P.tile([128, NCH, H * OWN], bf16, tag="cbt")
        for c in range(NCH):
            o4, w4 = 4 * offs[c], 4 * S[c]
            nc.gpsimd.local_scatter(
                out_ap=cbt[:, c, :],
                data_ap=wdataf[:, o4:o4 + w4],
                idxs_ap=pqt_t[:, o4:o4 + w4],
                channels=128, num_elems=H * OWN, num_idxs=w4)

        if not on("attn"):
            nc.sync.dma_start(out=out_d[:].rearrange("(t p) c -> p t c", p=128),
                              in_=outf[:])
            nc.compile()
            return nc
        # ---------------- attention ----------------
        av_ps = PSA.tile([DH + 1, H, OWN], fp32, tag="avps")
        for c in range(NCH):
            ksl = slice(128 * c, 128 * (c + 1))
            sps = PSB.tile([128, H, OWN], fp32, tag="sps", name="sps", bufs=2)
            for h in range(H):
                nc.tensor.matmul(sps[:, h, :], lhsT=Pk[:, ksl],
                                 rhs=qT4[:, h, :], start=True, stop=True)
            expS = WK.tile([128, OWN, H], bf16, tag="expS", bufs=3)
            nc.scalar.activation(
                expS[:].rearrange("p q h -> p h q"),
                sps[:],
                AF.Exp, scale=rstdsc[:, c:c + 1])
            punT = WK.tile([128, OWN, H], bf16, tag="punT", bufs=3)
            nc.vector.tensor_tensor(
                out=punT[:],
                in0=expS[:],
                in1=cbt[:, c, :].rearrange("p (q h) -> p q h", h=H),
                op=OP.mult)
            for h in range(H):
                nc.tensor.matmul(av_ps[:, h, :],
                                 lhsT=vsb[:, c, h, :],
                                 rhs=punT[:, :, h:h + 1],
                                 start=(c == 0), stop=(c == NCH - 1),
                                 skip_group_check=True)

        if not on("post"):
            nc.sync.dma_start(out=out_d[:].rearrange("(t p) c -> p t c", p=128),
                              in_=outf[:])
            nc.compile()
            return nc
        avsb = P.tile([DH + 1, H, OWN], bf16, tag="avsb")
        nc.vector.tensor_copy(out=avsb[:], in_=av_ps[:])

        # ---------------- attn normalize + output projection ----------------
        x2 = P.tile([128, QT, C], fp32, tag="x2")
        attn_n = P.tile([128, QT, C], bf16, tag="attn_n")
        for t in range(QT):
            qsl = slice(128 * t, 128 * (t + 1))
            for h in range(H):
                tps = PSB.tile([128, DH + 1], bf16, tag="big", name="tps")
                nc.tensor.transpose(out=tps[:], in_=avsb[:, h, qsl],
                                    identity=identb[:DH + 1, :DH + 1])
                rcol = WK.tile([128, 1], fp32, tag="rcol")
                nc.vector.reciprocal(rcol[:], tps[:, DH:DH + 1])
                nc.vector.tensor_scalar_mul(
                    attn_n[:, t, DH * h:DH * (h + 1)], tps[:, :DH], rcol[:])
        attnTn = P.tile([128, OWN], bf16, tag="attnTn")
        for t in range(QT):
            qsl = slice(128 * t, 128 * (t + 1))
            atps = PSB.tile([128, 128], bf16, tag="big", name="atps")
            nc.tensor.transpose(out=atps[:], in_=attn_n[:, t, :],
                                identity=identb[:])
            nc.vector.tensor_copy(out=attnTn[:, qsl], in_=atps[:])
        for t in range(QT):
            qsl = slice(128 * t, 128 * (t + 1))
            x2ps = PSB.tile([128, C], fp32, tag="big", name="x2ps")
            nc.tensor.matmul(x2ps[:], lhsT=attnTn[:, qsl], rhs=Wot[:],
                             start=True, stop=True)
            nc.vector.tensor_tensor(out=x2[:, t, :], in0=x2ps[:],
                                    in1=xow[:, t, :], op=OP.add)

        if not on("tail"):
            nc.sync.dma_start(out=out_d[:].rearrange("(t p) c -> p t c", p=128),
                              in_=outf[:])
            nc.compile()
            return nc
        # ---------------- LN2 + MLP ----------------
        h2T = P.tile([128, OWN], bf16, tag="h2T")
        for t in range(QT):
            mu = WK.tile([128, 1], fp32, tag="mu")
            nc.vector.tensor_reduce(out=mu[:], in_=x2[:, t, :], axis=AX.X,
                                    op=OP.add)
            nc.vector.tensor_scalar_mul(mu[:], mu[:], 1.0 / C)
            xc = WK.tile([128, C], fp32, tag="xc")
            nc.vector.tensor_scalar_sub(xc[:], x2[:, t, :], mu[:])
            sq = WK.tile([128, C], fp32, tag="sq")
            var = WK.tile([128, 1], fp32, tag="var")
            nc.scalar.activation(sq[:], xc[:], AF.Square, accum_out=var[:])
            std = WK.tile([128, 1], fp32, tag="std")
            nc.scalar.activation(std[:], var[:], AF.Sqrt, bias=epscol[:], scale=1.0 / C)
            rstd = WK.tile([128, 1], fp32, tag="rstd")
            nc.vector.reciprocal(rstd[:], std[:])
            hloc = WK.tile([128, C], bf16, tag="hloc")
            nc.vector.tensor_scalar_mul(hloc[:], xc[:], rstd[:])
            hps = PSB.tile([128, 128], bf16, tag="big", name="hps2")
            nc.tensor.transpose(out=hps[:], in_=hloc[:], identity=identb[:])
            nc.vector.tensor_copy(out=h2T[:, 128 * t:128 * (t + 1)], in_=hps[:])

        zT = P.tile([128, 4, OWN], bf16, tag="zT")
        for k in range(4):
            zps = PSB.tile([128, OWN], fp32, tag="big", name="zps")
            nc.tensor.matmul(zps[:], lhsT=w1t[:, k, :], rhs=h2T[:],
                             start=True, stop=True)
            sg = WK.tile([128, OWN], fp32, tag="sg")
            nc.scalar.activation(sg[:], zps[:], AF.Sigmoid,
                                 bias=b1c[:, k:k + 1])
            z1b = WK.tile([128, OWN], fp32, tag="z1b")
            nc.vector.tensor_scalar_add(z1b[:], zps[:], b1c[:, k:k + 1])
            nc.vector.tensor_tensor(out=zT[:, k, :], in0=z1b[:], in1=sg[:],
                                    op=OP.mult)

        for t in range(QT):
            qsl = slice(128 * t, 128 * (t + 1))
            ops = PSB.tile([128, C], fp32, tag="big", name="ops")
            for k in range(4):
                nc.tensor.matmul(ops[:], lhsT=zT[:, k, qsl], rhs=w2t[:, k, :],
                                 start=(k == 0), stop=False)
            nc.tensor.matmul(ops[:], lhsT=ones_row[:], rhs=b2row[:],
                             start=False, stop=True)
            nc.vector.tensor_tensor(out=outf[:, t, :], in0=ops[:],
                                    in1=x2[:, t, :], op=OP.add)
        nc.sync.dma_start(out=out_d[:].rearrange("(t p) c -> p t c", p=128),
                          in_=outf[:])

    nc.compile()
    return nc


# --------------------------------------------------------------------------
# Entry point
# --------------------------------------------------------------------------

def _make_in_maps(inputs, prep_tensors):
    x = np.ascontiguousarray(np.asarray(inputs["x"], np.float32))
    pair = np.asarray(inputs["pair_emb"], np.float32)
    maps = []
    for r in range(NCORES):
        t = prep_tensors[r]
        m = dict(
            x=x,
            xown=np.ascontiguousarray(x[r * OWN:(r + 1) * OWN]),
            pair=np.ascontiguousarray(
                pair[r * OWN:(r + 1) * OWN].reshape(OWN * N, CP)),
            Wq=np.asarray(inputs["Wq"], np.float32),
            Wk=np.asarray(inputs["Wk"], np.float32),
            Wv=np.asarray(inputs["Wv"], np.float32),
            Wout=np.asarray(inputs["Wout"], np.float32),
            W1=np.asarray(inputs["W1"], np.float32),
            W2=np.asarray(inputs["W2"], np.float32),
            b1=np.asarray(inputs["b1"], np.float32),
            b2=np.asarray(inputs["b2"], np.float32),
            Wb=np.ascontiguousarray(np.asarray(inputs["Wb"], np.float32)[:, 0]),
            pioff=t["pioff"], poff=t["poff"], pcnt=t["pcnt"], pqt=t["pqt"],
        )
        maps.append(m)
    return maps


def run(inputs, trace=False):
    import sys
    if "/opt/trn_rl_repo" not in sys.path:
        sys.path.insert(0, "/opt/trn_rl_repo")
    from concourse import bass_utils

    S, offs, Stot, prep_tensors = host_prep(inputs["block_index"])
    nc = build_nc(S, offs, Stot)
    in_maps = _make_in_maps(inputs, prep_tensors)
    res = bass_utils.run_bass_kernel_spmd(
        nc, in_maps, core_ids=list(range(NCORES)), trace=trace)
    out = np.concatenate([res.results[r]["out"] for r in range(NCORES)], axis=0)
    return out.astype(np.float32), res


def kernel(**inputs):
    out, _ = run(inputs)
    return out
